# revision 40
# baseline (speedup 1.0000x reference)
"""DeepseekOCR text MoE layer on 8 Trainium2 NeuronCores.

Expert-parallel: 4 routed experts per core (bucketed by token count so
every core's slot j has a similar load); shared expert tensor-sharded
over its intermediate dim (352 columns per core). Router + token
gather/scatter run on host (full-I/O contract).

Routed experts run in fp8 e4m3 with DoubleRow matmuls (256-deep
contraction per instruction, ~1.5x the bf16 PE rate); the shared
expert — which carries ~97% of the output energy — stays bf16, so the
total rel-err lands around 1.1e-2. Weight scales (gate x64, up x8,
down x64) keep fp8 operands out of the subnormal range; the gate scale
is divided back out inside the Silu activation and the up/down scales
are folded into the host-side combine weights.

Device program per core:
  phase A (per expert slot):  hT[h,c] = silu(wg.T @ xgT) * (wu.T @ xgT)
  phase B (per expert slot):  yT[d,c] = wd.T-tiles @ hT   (tokens on the
                              moving free dim -> no 128-token rounding)
  shared (per 512-token quarter): same silu-mlp with sharded bf16 weights.
Host: out = scatter_add(yT * combine_w) + sum_cores(ys).
"""

import numpy as np
import ml_dtypes

import concourse.bacc as bacc
import concourse.mybir as mybir
import concourse.tile as tile
from concourse.bass_utils import run_bass_kernel_spmd

B, S, D = 2, 1024, 2048
E, H, K = 32, 1408, 6
H_SHARED = 2816
ROUTED_SCALE = 1.0
T = B * S                      # 2048 tokens
N_CORES = 8
E_LOC = E // N_CORES           # 4 experts per core
HS_LOC = H_SHARED // N_CORES   # 352 shared-intermediate cols per core
HS_PAD = 384                   # padded to 3 k-tiles of 128
NH = H // 128                  # 11 h-tiles per routed expert
NH2 = 12                       # padded to 6 DoubleRow h-pairs
ND = D // 512                  # 4 d-groups (512 cols each)
NKD = D // 128                 # 16 contraction k-tiles over D
NKP = NKD // 2                 # 8 DoubleRow k-pairs over D
NSH = HS_PAD // 128            # 3 h-tiles for shared
TQ = 512                       # shared-expert token chunk
NTQ = T // TQ                  # 4 chunks

SWG = 64.0                     # fp8 scale on gate weights (undone in Silu)
SWU = 8.0                      # fp8 scale on up weights (h inherits it)
SWD = 64.0                     # fp8 scale on down weights
Y_DESCALE = 1.0 / (SWU * SWD)  # folded into host combine weights

BF16 = ml_dtypes.bfloat16
FP8 = ml_dtypes.float8_e4m3
f32 = mybir.dt.float32
bf16 = mybir.dt.bfloat16
fp8 = mybir.dt.float8e4
DR = mybir.MatmulPerfMode.DoubleRow

LAST_RESULTS = None            # BassKernelResults of the latest run (for test harness)


def _route(x, gate_w):
    """Greedy top-k softmax router, fp32 numpy (matches jax.lax.top_k order)."""
    logits = x @ gate_w.T                              # [T, E]
    m = logits.max(-1, keepdims=True)
    ex = np.exp(logits - m)
    scores = ex / ex.sum(-1, keepdims=True)
    topk_i = np.argsort(-scores, axis=-1, kind="stable")[:, :K]
    topk_w = np.take_along_axis(scores, topk_i, -1) * ROUTED_SCALE
    return topk_i, topk_w.astype(np.float32)


def _q8(a, s):
    return np.clip(np.asarray(a, np.float32) * s, -240, 240).astype(FP8)


def _expert_mlp(nc, pools, slabs, C, hT_tag):
    """Emit phase A (gate/up + silu*mul -> fp8 hT) for one expert."""
    psA, tmp_p, ht_p = pools
    xg, w_slabs = slabs                         # w_slabs[h] = (gate_slab, up_slab)
    NCC = -(-C // 512)
    hT = ht_p.tile([128, NH2, C], fp8, tag=hT_tag)
    nc.vector.memset(hT[:, NH, :], 0.0)         # zero the pad h-tile
    for h in range(NH):
        wg_s, wu_s = w_slabs[h]                 # each [128, NKD, 128] fp8
        for cc in range(NCC):
            w = min(512, C - cc * 512)
            cs = slice(cc * 512, cc * 512 + w)
            pg = psA.tile([128, w], f32, tag="psA")
            for k in range(NKP):
                nc.tensor.matmul(pg[:], wg_s[:, 2 * k:2 * k + 2],
                                 xg[:, 2 * k:2 * k + 2, cs],
                                 start=(k == 0), stop=(k == NKP - 1), perf_mode=DR)
            pu = psA.tile([128, w], f32, tag="psA")
            for k in range(NKP):
                nc.tensor.matmul(pu[:], wu_s[:, 2 * k:2 * k + 2],
                                 xg[:, 2 * k:2 * k + 2, cs],
                                 start=(k == 0), stop=(k == NKP - 1), perf_mode=DR)
            tmp = tmp_p.tile([128, 512], bf16, tag="tmp")
            nc.scalar.activation(tmp[:, :w], pg[:],
                                 mybir.ActivationFunctionType.Silu,
                                 scale=1.0 / SWG)
            nc.vector.tensor_mul(hT[:, h, cs], tmp[:, :w], pu[:])
    return hT


def _build_bass(Cs):
    """Per-core Tile program; Cs[j] = routed token capacity of expert slot j."""
    Cmax = max(Cs)
    nc = bacc.Bacc(None, target_bir_lowering=False)

    xgt = nc.dram_tensor("xgt", [E_LOC, 128, NKD, Cmax], fp8, kind="ExternalInput")
    wgu = nc.dram_tensor("wgu", [E_LOC, NH, 128, 2, NKD, 128], fp8, kind="ExternalInput")
    wdd = nc.dram_tensor("wdd", [E_LOC, ND * 2, 128, 2, NH2 // 2, 2, 128], fp8,
                         kind="ExternalInput")
    xtq = nc.dram_tensor("xtq", [NTQ, 128, NKD, TQ], bf16, kind="ExternalInput")
    swgu = nc.dram_tensor("swgu", [128, 2, NSH, NKD, 128], bf16, kind="ExternalInput")
    swdd = nc.dram_tensor("swdd", [128, ND, NSH, 512], bf16, kind="ExternalInput")
    y_out = nc.dram_tensor("y_out", [E_LOC, ND * 4, 128, Cmax], bf16, kind="ExternalOutput")
    ys_out = nc.dram_tensor("ys_out", [T, D], bf16, kind="ExternalOutput")

    with tile.TileContext(nc) as tc:
        with (
            tc.tile_pool(name="wgu_p", bufs=6) as wgu_p,
            tc.tile_pool(name="wd_p", bufs=8) as wd_p,
            tc.tile_pool(name="swgu_p", bufs=1) as swgu_p,
            tc.tile_pool(name="swd_p", bufs=1) as swd_p,
            tc.tile_pool(name="xg_p", bufs=2) as xg_p,
            tc.tile_pool(name="xt_p", bufs=3) as xt_p,
            tc.tile_pool(name="ht_p", bufs=2) as ht_p,
            tc.tile_pool(name="hst_p", bufs=2) as hst_p,
            tc.tile_pool(name="tmp_p", bufs=2) as tmp_p,
            tc.tile_pool(name="y_p", bufs=4) as y_p,
            tc.tile_pool(name="psA", bufs=4, space="PSUM") as psA,
            tc.tile_pool(name="psB", bufs=4, space="PSUM") as psB,
        ):
            sg_slabs, sd_slabs, xq_tiles = [], [], [None] * NTQ

            # PE warm-up on zeros while the first loads land (HAM un-throttle)
            warm = tmp_p.tile([128, 512], bf16, tag="tmp")
            nc.vector.memset(warm[:], 0.0)
            pwarm = psA.tile([128, 512], f32, tag="psA")
            for _ in range(12):
                nc.tensor.matmul(pwarm[:], warm[:, :128], warm[:], start=True, stop=True)

            def load_shared_gu():
                s = swgu_p.tile([128, 2, NSH, NKD, 128], bf16, tag="swgu")
                nc.sync.dma_start(s[:, 0], swgu[:, 0])
                nc.sync.dma_start(s[:, 1], swgu[:, 1])
                sg_slabs.append(s)
                xq_tiles[0] = xt_p.tile([128, NKD, TQ], bf16, tag="xt", name="xq0")
                nc.sync.dma_start(xq_tiles[0][:], xtq[0])

            def load_shared_down():
                s2 = swd_p.tile([128, ND, NSH, 512], bf16, tag="swd")
                nc.sync.dma_start(s2[:], swdd[:])
                sd_slabs.append(s2)

            hs_tiles = [None] * NTQ

            def shared_gu(q):
                """Gate/up + silu*mul for one 512-token slice of the shared
                expert. Runs between phase A and phase B of expert q so every
                PE phase boundary is free of intra-phase result dependencies."""
                xq = xq_tiles[q]
                for qn in (q + 1, q + 2):
                    if qn < NTQ and xq_tiles[qn] is None:
                        xq_tiles[qn] = xt_p.tile([128, NKD, TQ], bf16, tag="xt",
                                                 name=f"xq{qn}")
                        nc.sync.dma_start(xq_tiles[qn][:], xtq[qn])
                hsT = hst_p.tile([128, NSH, TQ], bf16, tag="hst")
                hs_tiles[q] = hsT
                sgu = sg_slabs[0]
                for h in range(NSH):
                    pg = psA.tile([128, TQ], f32, tag="psA")
                    for k in range(NKD):
                        nc.tensor.matmul(pg[:], sgu[:, 0, h, k], xq[:, k],
                                         start=(k == 0), stop=(k == NKD - 1))
                    pu = psA.tile([128, TQ], f32, tag="psA")
                    for k in range(NKD):
                        nc.tensor.matmul(pu[:], sgu[:, 1, h, k], xq[:, k],
                                         start=(k == 0), stop=(k == NKD - 1))
                    tmp = tmp_p.tile([128, 512], bf16, tag="tmp")
                    nc.scalar.activation(tmp[:, :TQ], pg[:],
                                         mybir.ActivationFunctionType.Silu)
                    nc.vector.tensor_mul(hsT[:, h, :], tmp[:, :TQ], pu[:])

            def shared_down(q):
                """Down-proj + store for one 512-token shared slice; one
                packed [128, D] store per 128-token row block (4KB lines)."""
                st_eng = nc.sync if q == NTQ - 1 else nc.scalar
                hsT = hs_tiles[q]
                for ci in range(TQ // 128):
                    yst = y_p.tile([128, ND, 512], bf16, tag="ys")
                    for d in range(ND):
                        py = psB.tile([128, 512], f32, tag="psB")
                        for h in range(NSH):
                            nc.tensor.matmul(py[:], hsT[:, h, ci * 128:(ci + 1) * 128],
                                             sd_slabs[0][:, d, h],
                                             start=(h == 0), stop=(h == NSH - 1))
                        nc.vector.tensor_copy(yst[:, d, :], py[:])
                    st_eng.dma_start(
                        ys_out[q * TQ + ci * 128:q * TQ + (ci + 1) * 128, :],
                        yst[:])

            # ---- routed experts, shared quarters interleaved as DMA slack ----
            def load_xg(j):
                # whole-expert load: one descriptor, contiguous 16*Cmax lines
                xc = xg_p.tile([128, NKD, Cmax], fp8, tag="xg", name=f"xg{j}")
                nc.sync.dma_start(xc[:], xgt[j])
                return xc

            def load_wgu_h0(j):
                gu = wgu_p.tile([128, 2, NKD, 128], fp8, tag="wgu", name=f"wgu{j}_h0")
                nc.sync.dma_start(gu[:, 0], wgu[j, 0, :, 0])
                nc.sync.dma_start(gu[:, 1], wgu[j, 0, :, 1])
                return gu

            xg_next = None
            wgu_h0_next = None
            for j in range(E_LOC):
                C = Cs[j]
                if j == 0:
                    # fan the first critical loads across three DMA queues in
                    # consumption order, quartering xg, so phase A(0)'s first
                    # psum group starts as soon as the earliest pieces land
                    # instead of gating on monolithic transfers on a cold ramp
                    xg_next = xg_p.tile([128, NKD, Cmax], fp8, tag="xg", name="xg0")
                    wgu_h0_next = wgu_p.tile([128, 2, NKD, 128], fp8,
                                             tag="wgu", name="wgu0_h0")
                    KQ = NKD // 4
                    nc.sync.dma_start(xg_next[:, :KQ], xgt[0, :, :KQ])
                    nc.sync.dma_start(xg_next[:, KQ:2 * KQ], xgt[0, :, KQ:2 * KQ])
                    nc.scalar.dma_start(wgu_h0_next[:, 0], wgu[0, 0, :, 0])
                    nc.scalar.dma_start(xg_next[:, 2 * KQ:3 * KQ],
                                        xgt[0, :, 2 * KQ:3 * KQ])
                    nc.scalar.dma_start(wgu_h0_next[:, 1], wgu[0, 0, :, 1])
                    nc.gpsimd.dma_start(xg_next[:, 3 * KQ:], xgt[0, :, 3 * KQ:])
                w_slabs = [(wgu_h0_next[:, 0], wgu_h0_next[:, 1])]
                for h in range(1, NH):
                    gu = wgu_p.tile([128, 2, NKD, 128], fp8, tag="wgu")
                    eng = nc.gpsimd if (j == 0 and h in (1, 2)) else nc.sync
                    eng.dma_start(gu[:], wgu[j, h])
                    w_slabs.append((gu[:, 0], gu[:, 1]))
                xg = xg_next
                hT = _expert_mlp(nc, (psA, tmp_p, ht_p),
                                 (xg, w_slabs), C, "ht")

                def load_wd(dp):
                    wd_s = wd_p.tile([128, 2, NH2 // 2, 2, 128], fp8, tag="wd")
                    nc.sync.dma_start(wd_s[:], wdd[j, dp])
                    return wd_s

                if j == 0:
                    # need order: shared-gu deps (for Q-gu(0)) before wd slabs
                    # (for B(0)) before next expert's inputs and swdd
                    load_shared_gu()
                    wd_slabs = [load_wd(dp) for dp in range(ND * 2)]
                    xg_next = load_xg(1)
                    wgu_h0_next = load_wgu_h0(1)
                    load_shared_down()
                else:
                    wd_slabs = [load_wd(dp) for dp in range(ND * 2)]
                    if j + 1 < E_LOC:
                        xg_next = load_xg(j + 1)
                        wgu_h0_next = load_wgu_h0(j + 1)

                shared_gu(j)
                if j == E_LOC - 1:
                    # last iteration: run the shared down-proj (big packed
                    # stores) BEFORE phase B so the kernel tail is only a
                    # small y store + drain
                    shared_down(j)

                # phase B: stationary = wd d-tiles (DR h-pairs), moving = hT tokens
                NCC = -(-C // 512)
                st_eng = nc.sync if j == E_LOC - 1 else nc.scalar
                for dp in range(ND * 2):
                    wd_s = wd_slabs[dp]
                    for dt in range(2):
                        for cc in range(NCC):
                            w = min(512, C - cc * 512)
                            cs = slice(cc * 512, cc * 512 + w)
                            py = psB.tile([128, 512], f32, tag="psB")
                            for hp in range(NH2 // 2):
                                nc.tensor.matmul(py[:, :w], wd_s[:, dt, hp],
                                                 hT[:, 2 * hp:2 * hp + 2, cs],
                                                 start=(hp == 0), stop=(hp == NH2 // 2 - 1),
                                                 perf_mode=DR)
                            yst = y_p.tile([128, 512], bf16, tag="y")
                            nc.vector.tensor_copy(yst[:, :w], py[:, :w])
                            st_eng.dma_start(
                                y_out[j, dp * 2 + dt, :, cs], yst[:, :w])
                if j != E_LOC - 1:
                    shared_down(j)
    nc.compile()
    return nc


def kernel(hidden_states, gate_w, wg, wu, wd, swg, swu, swd):
    global LAST_RESULTS
    x = np.ascontiguousarray(np.asarray(hidden_states, np.float32).reshape(T, D))
    gate_w = np.asarray(gate_w, np.float32)
    wg = np.asarray(wg, np.float32)
    wu = np.asarray(wu, np.float32)
    wd = np.asarray(wd, np.float32)
    swg = np.asarray(swg, np.float32)
    swu = np.asarray(swu, np.float32)
    swd = np.asarray(swd, np.float32)

    # ---- host router ----
    topk_i, topk_w = _route(x, gate_w)
    idx = [np.where((topk_i == e).any(-1))[0] for e in range(E)]
    wts = [(topk_w * (topk_i == e))[idx[e]].sum(-1).astype(np.float32) for e in range(E)]
    cnts = np.array([len(i) for i in idx])
    # bucket experts: slot j on every core serves similarly-loaded experts
    ranked = np.argsort(-cnts, kind="stable")            # expert ids, busiest first
    emap = ranked.reshape(E_LOC, N_CORES)                # emap[j, c] -> expert id
    Cs = [max(16, -(-int(cnts[emap[j]].max()) // 16) * 16) for j in range(E_LOC)]
    Cmax = max(Cs)

    nc = _build_bass(Cs)

    # ---- host shard + layout prep (all DMA sources partition-major) ----
    xT = np.ascontiguousarray(x.T)                      # [D, T] fp32
    xtq_np = np.ascontiguousarray(
        xT.reshape(NKD, 128, NTQ, TQ).transpose(2, 1, 0, 3).astype(BF16))

    in_maps = []
    for c in range(N_CORES):
        wgu_np = np.empty((E_LOC, NH, 128, 2, NKD, 128), FP8)
        wdd_np = np.empty((E_LOC, ND * 2, 128, 2, NH2 // 2, 2, 128), FP8)
        xgt_np = np.zeros((E_LOC, 128, NKD, Cmax), FP8)
        for j in range(E_LOC):
            e = int(emap[j, c])
            wgu_np[j] = (np.stack([_q8(wg[e], SWG), _q8(wu[e], SWU)])
                         .astype(np.float32)
                         .reshape(2, NKD, 128, NH, 128)
                         .transpose(3, 2, 0, 1, 4).astype(FP8))
            wd_pad = np.zeros((NH2 * 128, D), np.float32)
            wd_pad[:H] = wd[e] * SWD
            wdd_np[j] = (np.clip(wd_pad, -240, 240)
                         .reshape(NH2 // 2, 2, 128, ND * 2, 2, 128)
                         .transpose(3, 2, 4, 0, 1, 5).astype(FP8))
            cnt = int(cnts[e])
            xg = xT[:, idx[e]]                          # [D, cnt] fp32
            xgt_np[j, :, :, :cnt] = (np.clip(xg, -240, 240)
                                     .reshape(NKD, 128, cnt)
                                     .transpose(1, 0, 2).astype(FP8))
        sl = slice(c * HS_LOC, (c + 1) * HS_LOC)
        swg_c = np.zeros((D, HS_PAD), np.float32); swg_c[:, :HS_LOC] = swg[:, sl]
        swu_c = np.zeros((D, HS_PAD), np.float32); swu_c[:, :HS_LOC] = swu[:, sl]
        swd_c = np.zeros((HS_PAD, D), np.float32); swd_c[:HS_LOC] = swd[sl, :]
        swgu_np = (np.stack([swg_c, swu_c])
                   .reshape(2, NKD, 128, NSH, 128)
                   .transpose(2, 0, 3, 1, 4).astype(BF16))
        swdd_np = (swd_c.reshape(NSH, 128, ND, 512)
                   .transpose(1, 2, 0, 3).astype(BF16))
        in_maps.append({
            "xgt": np.ascontiguousarray(xgt_np),
            "wgu": np.ascontiguousarray(wgu_np),
            "wdd": np.ascontiguousarray(wdd_np),
            "xtq": xtq_np,
            "swgu": np.ascontiguousarray(swgu_np),
            "swdd": np.ascontiguousarray(swdd_np),
        })

    res = run_bass_kernel_spmd(nc, in_maps, core_ids=list(range(N_CORES)))
    LAST_RESULTS = res

    # ---- host unshard: scatter-add routed outputs, sum shared partials ----
    out = np.zeros((T, D), np.float32)
    for c in range(N_CORES):
        out += res.results[c]["ys_out"].astype(np.float32)
        yT = res.results[c]["y_out"]                    # [E_LOC, 16, 128, Cmax] bf16
        for j in range(E_LOC):
            e = int(emap[j, c])
            cnt = int(cnts[e])
            y = yT[j].reshape(D, Cmax)[:, :cnt].astype(np.float32)
            out[idx[e]] += (y * (wts[e] * Y_DESCALE)[None, :]).T
    return out.reshape(B, S, D)


# revision 41
# speedup vs baseline: 1.0117x; 1.0117x over previous
"""DeepseekOCR text MoE layer on 8 Trainium2 NeuronCores.

Expert-parallel: 4 routed experts per core (bucketed by token count so
every core's slot j has a similar load); shared expert tensor-sharded
over its intermediate dim (352 columns per core). Router + token
gather/scatter run on host (full-I/O contract).

Routed experts run in fp8 e4m3 with DoubleRow matmuls (256-deep
contraction per instruction, ~1.5x the bf16 PE rate); the shared
expert — which carries ~97% of the output energy — stays bf16, so the
total rel-err lands around 1.1e-2. Weight scales (gate x64, up x8,
down x64) keep fp8 operands out of the subnormal range; the gate scale
is divided back out inside the Silu activation and the up/down scales
are folded into the host-side combine weights.

Device program per core:
  phase A (per expert slot):  hT[h,c] = silu(wg.T @ xgT) * (wu.T @ xgT)
  phase B (per expert slot):  yT[d,c] = wd.T-tiles @ hT   (tokens on the
                              moving free dim -> no 128-token rounding)
  shared (per 512-token quarter): same silu-mlp with sharded bf16 weights.
Host: out = scatter_add(yT * combine_w) + sum_cores(ys).
"""

import numpy as np
import ml_dtypes

import concourse.bacc as bacc
import concourse.mybir as mybir
import concourse.tile as tile
from concourse.bass_utils import run_bass_kernel_spmd

B, S, D = 2, 1024, 2048
E, H, K = 32, 1408, 6
H_SHARED = 2816
ROUTED_SCALE = 1.0
T = B * S                      # 2048 tokens
N_CORES = 8
E_LOC = E // N_CORES           # 4 experts per core
HS_LOC = H_SHARED // N_CORES   # 352 shared-intermediate cols per core
HS_PAD = 384                   # padded to 3 k-tiles of 128
NH = H // 128                  # 11 h-tiles per routed expert
NH2 = 12                       # padded to 6 DoubleRow h-pairs
ND = D // 512                  # 4 d-groups (512 cols each)
NKD = D // 128                 # 16 contraction k-tiles over D
NKP = NKD // 2                 # 8 DoubleRow k-pairs over D
NSH = HS_PAD // 128            # 3 h-tiles for shared
TQ = 512                       # shared-expert token chunk
NTQ = T // TQ                  # 4 chunks

SWG = 64.0                     # fp8 scale on gate weights (undone in Silu)
SWU = 8.0                      # fp8 scale on up weights (h inherits it)
SWD = 64.0                     # fp8 scale on down weights
Y_DESCALE = 1.0 / (SWU * SWD)  # folded into host combine weights

BF16 = ml_dtypes.bfloat16
FP8 = ml_dtypes.float8_e4m3
f32 = mybir.dt.float32
bf16 = mybir.dt.bfloat16
fp8 = mybir.dt.float8e4
DR = mybir.MatmulPerfMode.DoubleRow

LAST_RESULTS = None            # BassKernelResults of the latest run (for test harness)


def _route(x, gate_w):
    """Greedy top-k softmax router, fp32 numpy (matches jax.lax.top_k order)."""
    logits = x @ gate_w.T                              # [T, E]
    m = logits.max(-1, keepdims=True)
    ex = np.exp(logits - m)
    scores = ex / ex.sum(-1, keepdims=True)
    topk_i = np.argsort(-scores, axis=-1, kind="stable")[:, :K]
    topk_w = np.take_along_axis(scores, topk_i, -1) * ROUTED_SCALE
    return topk_i, topk_w.astype(np.float32)


def _q8(a, s):
    return np.clip(np.asarray(a, np.float32) * s, -240, 240).astype(FP8)


def _expert_mlp(nc, pools, slabs, C, hT_tag):
    """Emit phase A (gate/up + silu*mul -> fp8 hT) for one expert."""
    psA, tmp_p, ht_p = pools
    xg, w_slabs = slabs                         # w_slabs[h] = (gate_slab, up_slab)
    NCC = -(-C // 512)
    hT = ht_p.tile([128, NH2, C], fp8, tag=hT_tag)
    nc.vector.memset(hT[:, NH, :], 0.0)         # zero the pad h-tile
    for h in range(NH):
        wg_s, wu_s = w_slabs[h]                 # each [128, NKD, 128] fp8
        for cc in range(NCC):
            w = min(512, C - cc * 512)
            cs = slice(cc * 512, cc * 512 + w)
            pg = psA.tile([128, w], f32, tag="psA")
            for k in range(NKP):
                nc.tensor.matmul(pg[:], wg_s[:, 2 * k:2 * k + 2],
                                 xg[:, 2 * k:2 * k + 2, cs],
                                 start=(k == 0), stop=(k == NKP - 1), perf_mode=DR)
            pu = psA.tile([128, w], f32, tag="psA")
            for k in range(NKP):
                nc.tensor.matmul(pu[:], wu_s[:, 2 * k:2 * k + 2],
                                 xg[:, 2 * k:2 * k + 2, cs],
                                 start=(k == 0), stop=(k == NKP - 1), perf_mode=DR)
            tmp = tmp_p.tile([128, 512], bf16, tag="tmp")
            nc.scalar.activation(tmp[:, :w], pg[:],
                                 mybir.ActivationFunctionType.Silu,
                                 scale=1.0 / SWG)
            nc.vector.tensor_mul(hT[:, h, cs], tmp[:, :w], pu[:])
    return hT


def _build_bass(Cs):
    """Per-core Tile program; Cs[j] = routed token capacity of expert slot j."""
    Cmax = max(Cs)
    nc = bacc.Bacc(None, target_bir_lowering=False)

    xgt = nc.dram_tensor("xgt", [E_LOC, 128, NKD, Cmax], fp8, kind="ExternalInput")
    wgu = nc.dram_tensor("wgu", [E_LOC, NH, 128, 2, NKD, 128], fp8, kind="ExternalInput")
    wdd = nc.dram_tensor("wdd", [E_LOC, ND * 2, 128, 2, NH2 // 2, 2, 128], fp8,
                         kind="ExternalInput")
    xtq = nc.dram_tensor("xtq", [NTQ, 128, NKD, TQ], bf16, kind="ExternalInput")
    swgu = nc.dram_tensor("swgu", [128, 2, NSH, NKD, 128], bf16, kind="ExternalInput")
    swdd = nc.dram_tensor("swdd", [128, ND, NSH, 512], bf16, kind="ExternalInput")
    y_out = nc.dram_tensor("y_out", [E_LOC, ND * 4, 128, Cmax], bf16, kind="ExternalOutput")
    ys_out = nc.dram_tensor("ys_out", [T, D], bf16, kind="ExternalOutput")

    with tile.TileContext(nc) as tc:
        with (
            tc.tile_pool(name="wgu_p", bufs=6) as wgu_p,
            tc.tile_pool(name="wd_p", bufs=8) as wd_p,
            tc.tile_pool(name="swgu_p", bufs=1) as swgu_p,
            tc.tile_pool(name="swd_p", bufs=1) as swd_p,
            tc.tile_pool(name="xg_p", bufs=2) as xg_p,
            tc.tile_pool(name="xt_p", bufs=3) as xt_p,
            tc.tile_pool(name="ht_p", bufs=2) as ht_p,
            tc.tile_pool(name="hst_p", bufs=2) as hst_p,
            tc.tile_pool(name="tmp_p", bufs=2) as tmp_p,
            tc.tile_pool(name="y_p", bufs=4) as y_p,
            tc.tile_pool(name="psA", bufs=4, space="PSUM") as psA,
            tc.tile_pool(name="psB", bufs=4, space="PSUM") as psB,
        ):
            sg_slabs, sd_slabs, xq_tiles = [], [], [None] * NTQ

            # PE warm-up on zeros while the first loads land (HAM un-throttle)
            warm = tmp_p.tile([128, 512], bf16, tag="tmp")
            nc.vector.memset(warm[:], 0.0)
            pwarm = psA.tile([128, 512], f32, tag="psA")
            for _ in range(12):
                nc.tensor.matmul(pwarm[:], warm[:, :128], warm[:], start=True, stop=True)

            def load_shared_gu():
                s = swgu_p.tile([128, 2, NSH, NKD, 128], bf16, tag="swgu")
                nc.sync.dma_start(s[:, 0], swgu[:, 0])
                nc.sync.dma_start(s[:, 1], swgu[:, 1])
                sg_slabs.append(s)
                xq_tiles[0] = xt_p.tile([128, NKD, TQ], bf16, tag="xt", name="xq0")
                nc.sync.dma_start(xq_tiles[0][:], xtq[0])

            def load_shared_down():
                s2 = swd_p.tile([128, ND, NSH, 512], bf16, tag="swd")
                nc.sync.dma_start(s2[:], swdd[:])
                sd_slabs.append(s2)

            hs_tiles = [None] * NTQ

            def shared_gu(q):
                """Gate/up + silu*mul for one 512-token slice of the shared
                expert. Runs between phase A and phase B of expert q so every
                PE phase boundary is free of intra-phase result dependencies."""
                xq = xq_tiles[q]
                for qn in (q + 1, q + 2):
                    if qn < NTQ and xq_tiles[qn] is None:
                        xq_tiles[qn] = xt_p.tile([128, NKD, TQ], bf16, tag="xt",
                                                 name=f"xq{qn}")
                        nc.sync.dma_start(xq_tiles[qn][:], xtq[qn])
                hsT = hst_p.tile([128, NSH, TQ], bf16, tag="hst")
                hs_tiles[q] = hsT
                sgu = sg_slabs[0]
                for h in range(NSH):
                    pg = psA.tile([128, TQ], f32, tag="psA")
                    for k in range(NKD):
                        nc.tensor.matmul(pg[:], sgu[:, 0, h, k], xq[:, k],
                                         start=(k == 0), stop=(k == NKD - 1))
                    pu = psA.tile([128, TQ], f32, tag="psA")
                    for k in range(NKD):
                        nc.tensor.matmul(pu[:], sgu[:, 1, h, k], xq[:, k],
                                         start=(k == 0), stop=(k == NKD - 1))
                    tmp = tmp_p.tile([128, 512], bf16, tag="tmp")
                    nc.scalar.activation(tmp[:, :TQ], pg[:],
                                         mybir.ActivationFunctionType.Silu)
                    nc.vector.tensor_mul(hsT[:, h, :], tmp[:, :TQ], pu[:])

            def shared_down(q):
                """Down-proj + store for one 512-token shared slice; one
                packed [128, D] store per 128-token row block (4KB lines)."""
                st_eng = nc.sync if q == NTQ - 1 else nc.scalar
                hsT = hs_tiles[q]
                for ci in range(TQ // 128):
                    yst = y_p.tile([128, ND, 512], bf16, tag="ys")
                    for d in range(ND):
                        py = psB.tile([128, 512], f32, tag="psB")
                        for h in range(NSH):
                            nc.tensor.matmul(py[:], hsT[:, h, ci * 128:(ci + 1) * 128],
                                             sd_slabs[0][:, d, h],
                                             start=(h == 0), stop=(h == NSH - 1))
                        nc.vector.tensor_copy(yst[:, d, :], py[:])
                    st_eng.dma_start(
                        ys_out[q * TQ + ci * 128:q * TQ + (ci + 1) * 128, :],
                        yst[:])

            # ---- routed experts, shared quarters interleaved as DMA slack ----
            def load_xg(j):
                # whole-expert load: one descriptor, contiguous 16*Cmax lines
                xc = xg_p.tile([128, NKD, Cmax], fp8, tag="xg", name=f"xg{j}")
                nc.sync.dma_start(xc[:], xgt[j])
                return xc

            def load_wgu_h0(j):
                gu = wgu_p.tile([128, 2, NKD, 128], fp8, tag="wgu", name=f"wgu{j}_h0")
                nc.sync.dma_start(gu[:, 0], wgu[j, 0, :, 0])
                nc.sync.dma_start(gu[:, 1], wgu[j, 0, :, 1])
                return gu

            xg_next = None
            wgu_h0_next = None
            for j in range(E_LOC):
                C = Cs[j]
                if j == 0:
                    # fan the first critical loads across three DMA queues and
                    # split them into halves, so phase A(0)'s first psum group
                    # starts as soon as the earliest pieces land instead of
                    # gating on whole monolithic transfers on a cold DMA ramp
                    xg_next = xg_p.tile([128, NKD, Cmax], fp8, tag="xg", name="xg0")
                    nc.sync.dma_start(xg_next[:, :NKD // 2], xgt[0, :, :NKD // 2])
                    nc.sync.dma_start(xg_next[:, NKD // 2:], xgt[0, :, NKD // 2:])
                    wgu_h0_next = wgu_p.tile([128, 2, NKD, 128], fp8,
                                             tag="wgu", name="wgu0_h0")
                    nc.scalar.dma_start(wgu_h0_next[:, 0], wgu[0, 0, :, 0])
                    nc.scalar.dma_start(wgu_h0_next[:, 1], wgu[0, 0, :, 1])
                w_slabs = [(wgu_h0_next[:, 0], wgu_h0_next[:, 1])]
                for h in range(1, NH):
                    gu = wgu_p.tile([128, 2, NKD, 128], fp8, tag="wgu")
                    eng = nc.gpsimd if (j == 0 and h in (1, 2)) else nc.sync
                    eng.dma_start(gu[:], wgu[j, h])
                    w_slabs.append((gu[:, 0], gu[:, 1]))
                xg = xg_next
                hT = _expert_mlp(nc, (psA, tmp_p, ht_p),
                                 (xg, w_slabs), C, "ht")

                def load_wd(dp):
                    wd_s = wd_p.tile([128, 2, NH2 // 2, 2, 128], fp8, tag="wd")
                    nc.sync.dma_start(wd_s[:], wdd[j, dp])
                    return wd_s

                if j == 0:
                    # need order: shared-gu deps (for Q-gu(0)) before wd slabs
                    # (for B(0)) before next expert's inputs and swdd
                    load_shared_gu()
                    wd_slabs = [load_wd(dp) for dp in range(ND * 2)]
                    xg_next = load_xg(1)
                    wgu_h0_next = load_wgu_h0(1)
                    load_shared_down()
                else:
                    wd_slabs = [load_wd(dp) for dp in range(ND * 2)]
                    if j + 1 < E_LOC:
                        xg_next = load_xg(j + 1)
                        wgu_h0_next = load_wgu_h0(j + 1)

                shared_gu(j)
                if j == E_LOC - 1:
                    # last iteration: run the shared down-proj (big packed
                    # stores) BEFORE phase B so the kernel tail is only a
                    # small y store + drain
                    shared_down(j)

                # phase B: stationary = wd d-tiles (DR h-pairs), moving = hT tokens
                NCC = -(-C // 512)
                st_eng = nc.sync if j == E_LOC - 1 else nc.scalar
                for dp in range(ND * 2):
                    wd_s = wd_slabs[dp]
                    for dt in range(2):
                        for cc in range(NCC):
                            w = min(512, C - cc * 512)
                            cs = slice(cc * 512, cc * 512 + w)
                            py = psB.tile([128, 512], f32, tag="psB")
                            for hp in range(NH2 // 2):
                                nc.tensor.matmul(py[:, :w], wd_s[:, dt, hp],
                                                 hT[:, 2 * hp:2 * hp + 2, cs],
                                                 start=(hp == 0), stop=(hp == NH2 // 2 - 1),
                                                 perf_mode=DR)
                            yst = y_p.tile([128, 512], bf16, tag="y")
                            nc.vector.tensor_copy(yst[:, :w], py[:, :w])
                            st_eng.dma_start(
                                y_out[j, dp * 2 + dt, :, cs], yst[:, :w])
                if j != E_LOC - 1:
                    shared_down(j)
    nc.compile()
    return nc


def kernel(hidden_states, gate_w, wg, wu, wd, swg, swu, swd):
    global LAST_RESULTS
    x = np.ascontiguousarray(np.asarray(hidden_states, np.float32).reshape(T, D))
    gate_w = np.asarray(gate_w, np.float32)
    wg = np.asarray(wg, np.float32)
    wu = np.asarray(wu, np.float32)
    wd = np.asarray(wd, np.float32)
    swg = np.asarray(swg, np.float32)
    swu = np.asarray(swu, np.float32)
    swd = np.asarray(swd, np.float32)

    # ---- host router ----
    topk_i, topk_w = _route(x, gate_w)
    idx = [np.where((topk_i == e).any(-1))[0] for e in range(E)]
    wts = [(topk_w * (topk_i == e))[idx[e]].sum(-1).astype(np.float32) for e in range(E)]
    cnts = np.array([len(i) for i in idx])
    # bucket experts: slot j on every core serves similarly-loaded experts
    ranked = np.argsort(-cnts, kind="stable")            # expert ids, busiest first
    emap = ranked.reshape(E_LOC, N_CORES)                # emap[j, c] -> expert id
    Cs = [max(16, -(-int(cnts[emap[j]].max()) // 16) * 16) for j in range(E_LOC)]
    Cmax = max(Cs)

    nc = _build_bass(Cs)

    # ---- host shard + layout prep (all DMA sources partition-major) ----
    xT = np.ascontiguousarray(x.T)                      # [D, T] fp32
    xtq_np = np.ascontiguousarray(
        xT.reshape(NKD, 128, NTQ, TQ).transpose(2, 1, 0, 3).astype(BF16))

    in_maps = []
    for c in range(N_CORES):
        wgu_np = np.empty((E_LOC, NH, 128, 2, NKD, 128), FP8)
        wdd_np = np.empty((E_LOC, ND * 2, 128, 2, NH2 // 2, 2, 128), FP8)
        xgt_np = np.zeros((E_LOC, 128, NKD, Cmax), FP8)
        for j in range(E_LOC):
            e = int(emap[j, c])
            wgu_np[j] = (np.stack([_q8(wg[e], SWG), _q8(wu[e], SWU)])
                         .astype(np.float32)
                         .reshape(2, NKD, 128, NH, 128)
                         .transpose(3, 2, 0, 1, 4).astype(FP8))
            wd_pad = np.zeros((NH2 * 128, D), np.float32)
            wd_pad[:H] = wd[e] * SWD
            wdd_np[j] = (np.clip(wd_pad, -240, 240)
                         .reshape(NH2 // 2, 2, 128, ND * 2, 2, 128)
                         .transpose(3, 2, 4, 0, 1, 5).astype(FP8))
            cnt = int(cnts[e])
            xg = xT[:, idx[e]]                          # [D, cnt] fp32
            xgt_np[j, :, :, :cnt] = (np.clip(xg, -240, 240)
                                     .reshape(NKD, 128, cnt)
                                     .transpose(1, 0, 2).astype(FP8))
        sl = slice(c * HS_LOC, (c + 1) * HS_LOC)
        swg_c = np.zeros((D, HS_PAD), np.float32); swg_c[:, :HS_LOC] = swg[:, sl]
        swu_c = np.zeros((D, HS_PAD), np.float32); swu_c[:, :HS_LOC] = swu[:, sl]
        swd_c = np.zeros((HS_PAD, D), np.float32); swd_c[:HS_LOC] = swd[sl, :]
        swgu_np = (np.stack([swg_c, swu_c])
                   .reshape(2, NKD, 128, NSH, 128)
                   .transpose(2, 0, 3, 1, 4).astype(BF16))
        swdd_np = (swd_c.reshape(NSH, 128, ND, 512)
                   .transpose(1, 2, 0, 3).astype(BF16))
        in_maps.append({
            "xgt": np.ascontiguousarray(xgt_np),
            "wgu": np.ascontiguousarray(wgu_np),
            "wdd": np.ascontiguousarray(wdd_np),
            "xtq": xtq_np,
            "swgu": np.ascontiguousarray(swgu_np),
            "swdd": np.ascontiguousarray(swdd_np),
        })

    res = run_bass_kernel_spmd(nc, in_maps, core_ids=list(range(N_CORES)))
    LAST_RESULTS = res

    # ---- host unshard: scatter-add routed outputs, sum shared partials ----
    out = np.zeros((T, D), np.float32)
    for c in range(N_CORES):
        out += res.results[c]["ys_out"].astype(np.float32)
        yT = res.results[c]["y_out"]                    # [E_LOC, 16, 128, Cmax] bf16
        for j in range(E_LOC):
            e = int(emap[j, c])
            cnt = int(cnts[e])
            y = yT[j].reshape(D, Cmax)[:, :cnt].astype(np.float32)
            out[idx[e]] += (y * (wts[e] * Y_DESCALE)[None, :]).T
    return out.reshape(B, S, D)
